# revision 8
# baseline (speedup 1.0000x reference)
"""AttnBlock2D Trainium2 kernel.

Reference computation (per batch element b):
    q = Wq @ x_self + bq            (1x1 conv == per-pixel linear)
    k = Wk @ x_cross + bk
    v = Wv @ x_cross + bv
    per head h (8 heads, head_dim 64, n = 32*32 = 1024 pixels):
        scores = q_h^T k_h / 8      softmax over k-pixels
        o_h = attn @ v_h
    y = Wout @ o + bout + x_self

Sharding: pure data-parallel over batch; B == 8 == n_cores, each NeuronCore
computes one batch element end-to-end with replicated weights. No collectives.

On-device layout (per core):
    x_self, x_cross : [C=512, N=1024]   (channels on partitions)
    Q, K            : [512, 1024]       q/k channel-major (head h rows h*64..)
    VT_aug          : [N=1024, 8*(64+1)] v transposed, per-head 64 cols + ones
                      column (ones column makes the U matmul also emit the
                      softmax denominator as output row 64)
    scores^T        : [m=1024, n=1024] per head, m on partitions -> softmax
                      denominator computed by PE via the ones column; exp on ACT
    U = [v|1]^T E   : [65, 1024] psum; row 64 = sum_m exp(scores^T[m, n])
    O = U[0:64] / S : normalize via base-0 S hop + reciprocal + gpsimd bcast
    y = WoutT^T O + bout' + x_self,  bout' = bout + Wout@bv (folded on host)

bk is dropped: it shifts every score of a softmax row by the same constant
(softmax invariant). bv is folded into bout' because attention rows sum to 1.

Head pairs (2p, 2p+1) share Q/K row-tiles; their K=64 score matmuls are issued
back-to-back at partition bases 0/64 so the PE runs them concurrently in
disjoint row groups. The attention loop is software-pipelined one pair ahead:
while pair p's U matmuls accumulate (m-tile at a time), pair p+1's scores and
exps stream, keeping both PE and ACT dense.

Numerics knobs (env):
    ATT_MM = f32r | f32 | bf16   dtype of projection/score matmuls
    ATT_VE = bf16 | f32r | f32   dtype of V/E/O/out-proj matmul path
"""

import os
from contextlib import ExitStack

import ml_dtypes
import numpy as np

import concourse.bass as bass
import concourse.tile as tile
from concourse import bacc, mybir

# Problem dims (fixed by the harness problem)
B = 8
C = 512  # QUERY_DIM == CROSS_DIM == INNER
HEADS = 8
HD = 64
N = 1024  # 32*32 pixels
N_CORES = 8
HDP = HD + 1  # per-head cols in VT_aug (64 v-cols + 1 ones col)

F32 = mybir.dt.float32
F32R = mybir.dt.float32r
BF16 = mybir.dt.bfloat16


def _storage(dt_name):
    if dt_name == "bf16":
        return BF16
    if dt_name == "f32r":
        return F32R
    return F32


def _np_storage(dt_name):
    return ml_dtypes.bfloat16 if dt_name == "bf16" else np.float32


def build(mm="f32r", ve="bf16"):
    nc = bacc.Bacc(
        "TRN2", target_bir_lowering=False, debug=False, num_devices=N_CORES
    )
    mdt = _storage(mm)  # x, Wq/Wk/Wv, Q, K storage
    vdt = _storage(ve)  # VT_aug, E, O, WoutT storage

    xs_d = nc.dram_tensor("x_self", [C, N], mdt, kind="ExternalInput").ap()
    xc_d = nc.dram_tensor("x_cross", [C, N], mdt, kind="ExternalInput").ap()
    wq_d = nc.dram_tensor("wqT", [C, C], mdt, kind="ExternalInput").ap()
    wk_d = nc.dram_tensor("wkT", [C, C], mdt, kind="ExternalInput").ap()
    wv_d = nc.dram_tensor("wvT", [C, C], mdt, kind="ExternalInput").ap()
    wo_d = nc.dram_tensor("woutT", [C, C], vdt, kind="ExternalInput").ap()
    bq_d = nc.dram_tensor("bq", [C], F32, kind="ExternalInput").ap()
    bo_d = nc.dram_tensor("bout2", [C], F32, kind="ExternalInput").ap()
    need_resid = mm == "bf16"
    if need_resid:
        rs_d = nc.dram_tensor("resid", [C, N], F32, kind="ExternalInput").ap()
    y_d = nc.dram_tensor("y", [C, N], F32, kind="ExternalOutput").ap()

    MUL = mybir.AluOpType.mult
    ADD = mybir.AluOpType.add
    EXP = mybir.ActivationFunctionType.Exp

    with tile.TileContext(nc) as tc, ExitStack() as ctx:
        persist = ctx.enter_context(tc.tile_pool(name="persist", bufs=1))
        ppool = ctx.enter_context(tc.tile_pool(name="psum", bufs=1, space="PSUM"))
        epool = ctx.enter_context(
            tc.tile_pool(name="epool", bufs=18 if vdt == BF16 else 9)
        )
        npool = ctx.enter_context(tc.tile_pool(name="norm", bufs=2))
        ypool = ctx.enter_context(tc.tile_pool(name="yout", bufs=2))

        def load(name, src, shape, dtype):
            t = persist.tile(shape, dtype, tag=name, name=name)
            nc.sync.dma_start(t[:], src)
            return t

        # ---- persistent loads (Q-projection-critical tensors first) ------
        wq_s = [load(f"wq{i}", wq_d[i * 128 : (i + 1) * 128, :], [128, C], mdt)
                for i in range(4)]
        xs_s = [load(f"xs{i}", xs_d[i * 128 : (i + 1) * 128, :], [128, N], mdt)
                for i in range(4)]
        wk_s = [load(f"wk{i}", wk_d[i * 128 : (i + 1) * 128, :], [128, C], mdt)
                for i in range(4)]
        xc_s = [load(f"xc{i}", xc_d[i * 128 : (i + 1) * 128, :], [128, N], mdt)
                for i in range(4)]
        wv_s = [load(f"wv{i}", wv_d[i * 128 : (i + 1) * 128, :], [128, C], mdt)
                for i in range(4)]
        bq_s = load("bq", bq_d.rearrange("(a p) -> p a", p=128), [128, 4], F32)
        wo_s = [load(f"wo{i}", wo_d[i * 128 : (i + 1) * 128, :], [128, C], vdt)
                for i in range(4)]
        bo_s = load("bo", bo_d.rearrange("(a p) -> p a", p=128), [128, 4], F32)
        if need_resid:
            rs_s = [load(f"rs{i}", rs_d[i * 128 : (i + 1) * 128, :], [128, N], F32)
                    for i in range(4)]
        elif mm == "f32r":
            # f32r storage holds full fp32 bits; view as fp32 for the residual
            rs_s = None
        else:
            rs_s = xs_s

        # VT_aug tiles: per-head [64 v-cols | ones] blocks
        vt_s = [persist.tile([128, HEADS * HDP], vdt, tag=f"vt{t}", name=f"vt{t}")
                for t in range(8)]
        for t in range(8):
            ones_cols = vt_s[t][:].rearrange("p (h d) -> p h d", d=HDP)[
                :, :, HD : HD + 1
            ]
            nc.gpsimd.memset(ones_cols, 1.0)

        # ---- Q / K projections ------------------------------------------
        q_s, k_s = [], []
        for which, w_s, x_in, dst in (("q", wq_s, xs_s, q_s), ("k", wk_s, xc_s, k_s)):
            for mt in range(4):
                ps = ppool.tile([128, N], F32, tag="u", name="ps", bufs=2)
                for nh in range(2):
                    for ct in range(4):
                        nc.tensor.matmul(
                            ps[:, nh * 512 : (nh + 1) * 512],
                            lhsT=w_s[ct][:, mt * 128 : (mt + 1) * 128],
                            rhs=x_in[ct][:, nh * 512 : (nh + 1) * 512],
                            start=(ct == 0),
                            stop=(ct == 3),
                        )
                t = persist.tile([128, N], mdt, tag=f"{which}{mt}",
                                 name=f"{which}{mt}")
                if which == "q":
                    nc.vector.tensor_scalar_add(t[:], ps[:], bq_s[:, mt : mt + 1])
                else:
                    nc.vector.tensor_copy(out=t[:], in_=ps[:])
                dst.append(t)

        # ---- V^T projection (x_cross^T @ Wv^T) --------------------------
        for t in range(8):
            ps = ppool.tile([128, 512], F32, tag="sc", name="ps", bufs=2)
            for ct in range(4):
                nc.tensor.matmul(
                    ps[:, 0:512],
                    lhsT=xc_s[ct][:, t * 128 : (t + 1) * 128],
                    rhs=wv_s[ct][:, :],
                    start=(ct == 0),
                    stop=(ct == 3),
                )
            src = ps[:, 0:512].rearrange("p (h d) -> p h d", d=HD)
            dst = vt_s[t][:].rearrange("p (h d) -> p h d", d=HDP)[:, :, 0:HD]
            nc.vector.tensor_copy(out=dst, in_=src)

        # ---- attention: pair-pipelined ----------------------------------
        o_s = [persist.tile([128, N], vdt, tag=f"o{i}", name=f"o{i}")
               for i in range(4)]

        def scores_step(p, t):
            """scores + exp for heads (2p, 2p+1) at m-tile t.
            The two K=64 matmuls per n-half sit at partition bases 0/64 and are
            issued back-to-back -> concurrent PE row groups."""
            es = []
            for base in (0, 64):
                ps = ppool.tile([128, N], F32, tag="sc", name="sc", bufs=2)
                for nh in range(2):
                    nc.tensor.matmul(
                        ps[:, nh * 512 : (nh + 1) * 512],
                        lhsT=k_s[p][base : base + 64, t * 128 : (t + 1) * 128],
                        rhs=q_s[p][base : base + 64, nh * 512 : (nh + 1) * 512],
                        start=True,
                        stop=True,
                    )
                e = epool.tile([128, N], vdt, tag="e", name="e")
                nc.scalar.activation(e[:], ps[:], EXP, scale=0.125)
                es.append(e)
            return es

        def u_step(p, ups_pair, e_pair, t):
            """U accumulation for heads (2p, 2p+1) over m-tile t."""
            for i, h in enumerate((2 * p, 2 * p + 1)):
                for nh in range(2):
                    nc.tensor.matmul(
                        ups_pair[i][0:65, nh * 512 : (nh + 1) * 512],
                        lhsT=vt_s[t][:, h * HDP : (h + 1) * HDP],
                        rhs=e_pair[i][t][:, nh * 512 : (nh + 1) * 512],
                        start=(t == 0),
                        stop=(t == 7),
                    )

        def normalize(p, ups_pair):
            """O_h = U[0:64] / broadcast(U[64]) for heads (2p, 2p+1)."""
            for i, h in enumerate((2 * p, 2 * p + 1)):
                ups = ups_pair[i]
                base = (h % 2) * 64
                s0 = npool.tile([1, N], F32, tag="s0", name="s0")
                nc.vector.tensor_copy(out=s0[:], in_=ups[64:65, :])
                r0 = npool.tile([1, N], F32, tag="r0", name="r0")
                nc.vector.reciprocal_approx_fast(r0[:], s0[:])
                R = npool.tile([64, N], F32, tag="R", name="R")
                nc.gpsimd.partition_broadcast(R[:], r0[:])
                if base == 0:
                    nc.vector.tensor_tensor(
                        o_s[p][0:64, :], ups[0:64, :], R[:], op=MUL
                    )
                else:
                    stg = npool.tile([64, N], vdt, tag="stg", name="stg")
                    nc.vector.tensor_tensor(stg[:], ups[0:64, :], R[:], op=MUL)
                    nc.vector.tensor_copy(out=o_s[p][64:128, :], in_=stg[:])

        # prologue: scores for pair 0
        e_prev = [[], []]
        for t in range(8):
            ea, eb = scores_step(0, t)
            e_prev[0].append(ea)
            e_prev[1].append(eb)

        for p in range(4):
            ups_pair = [
                ppool.tile([128, N], F32, tag="u", name="ua", bufs=2),
                ppool.tile([128, N], F32, tag="u", name="ub", bufs=2),
            ]
            e_next = [[], []]
            for t in range(8):
                if p + 1 < 4:
                    ea, eb = scores_step(p + 1, t)
                    e_next[0].append(ea)
                    e_next[1].append(eb)
                u_step(p, ups_pair, e_prev, t)
            normalize(p, ups_pair)
            e_prev = e_next

        # ---- output projection + bias + residual ------------------------
        for mt in range(4):
            ps = ppool.tile([128, N], F32, tag="u", name="ps", bufs=2)
            for nh in range(2):
                for dt_ in range(4):
                    nc.tensor.matmul(
                        ps[:, nh * 512 : (nh + 1) * 512],
                        lhsT=wo_s[dt_][:, mt * 128 : (mt + 1) * 128],
                        rhs=o_s[dt_][:, nh * 512 : (nh + 1) * 512],
                        start=(dt_ == 0),
                        stop=(dt_ == 3),
                    )
            y = ypool.tile([128, N], F32, tag="y", name="y_t")
            if rs_s is None:
                resid_ap = xs_s[mt][:].bitcast(F32)
            else:
                resid_ap = rs_s[mt][:]
            nc.vector.scalar_tensor_tensor(
                y[:], ps[:], bo_s[:, mt : mt + 1], resid_ap, op0=ADD, op1=ADD
            )
            nc.sync.dma_start(y_d[mt * 128 : (mt + 1) * 128, :], y[:])

    nc.compile()
    return nc


_CACHE = {}


def get_nc(mm=None, ve=None):
    mm = mm or os.environ.get("ATT_MM", "f32r")
    ve = ve or os.environ.get("ATT_VE", "bf16")
    key = (mm, ve)
    if key not in _CACHE:
        _CACHE[key] = build(*key)
    return _CACHE[key], key


def make_in_maps(self_feature, cross_feature, Wq, bq, Wk, bk, Wv, bv, Wout, bout,
                 mm, ve):
    f32 = np.float32
    np_m = _np_storage(mm)
    np_v = _np_storage(ve)
    sf = np.asarray(self_feature, f32).reshape(B, C, N)
    cf = np.asarray(cross_feature, f32).reshape(B, C, N)
    Wq = np.asarray(Wq, f32)
    Wk = np.asarray(Wk, f32)
    Wv = np.asarray(Wv, f32)
    Wout = np.asarray(Wout, f32)
    wqT = np.ascontiguousarray(Wq.T).astype(np_m)
    wkT = np.ascontiguousarray(Wk.T).astype(np_m)
    wvT = np.ascontiguousarray(Wv.T).astype(np_m)
    woT = np.ascontiguousarray(Wout.T).astype(np_v)
    bq = np.asarray(bq, f32)
    bout2 = (np.asarray(bout, f32) + Wout @ np.asarray(bv, f32)).astype(f32)
    # bk is intentionally unused: adding k-bias shifts all scores in a softmax
    # row by the same amount, which softmax cancels exactly.
    del bk
    in_maps = []
    for b in range(B):
        m = {
            "x_self": np.ascontiguousarray(sf[b]).astype(np_m),
            "x_cross": np.ascontiguousarray(cf[b]).astype(np_m),
            "wqT": wqT, "wkT": wkT, "wvT": wvT, "woutT": woT,
            "bq": bq, "bout2": bout2,
        }
        if mm == "bf16":
            m["resid"] = np.ascontiguousarray(sf[b])
        in_maps.append(m)
    return in_maps


def kernel(self_feature, cross_feature, Wq, bq, Wk, bk, Wv, bv, Wout, bout):
    from concourse.bass_utils import run_bass_kernel_spmd

    nc, (mm, ve) = get_nc()
    in_maps = make_in_maps(self_feature, cross_feature, Wq, bq, Wk, bk, Wv, bv,
                           Wout, bout, mm, ve)
    res = run_bass_kernel_spmd(nc, in_maps, core_ids=list(range(N_CORES)))
    y = np.stack([res.results[b]["y"].reshape(C, 32, 32) for b in range(B)])
    return np.ascontiguousarray(y.astype(np.float32))


# revision 9
# speedup vs baseline: 1.2305x; 1.2305x over previous
"""AttnBlock2D Trainium2 kernel.

Reference computation (per batch element b):
    q = Wq @ x_self + bq            (1x1 conv == per-pixel linear)
    k = Wk @ x_cross + bk
    v = Wv @ x_cross + bv
    per head h (8 heads, head_dim 64, n = 32*32 = 1024 pixels):
        scores = q_h^T k_h / 8      softmax over k-pixels
        o_h = attn @ v_h
    y = Wout @ o + bout + x_self

Sharding: pure data-parallel over batch; B == 8 == n_cores, each NeuronCore
computes one batch element end-to-end with replicated weights. No collectives.

On-device layout (per core):
    x_self, x_cross : [C=512, N=1024]   (channels on partitions)
    Q, K            : [512, 1024]       q/k channel-major (head h rows h*64..)
    VT_aug          : [N=1024, 8*(64+1)] v transposed, per-head 64 cols + ones
                      column (ones column makes the U matmul also emit the
                      softmax denominator as output row 64)
    scores^T        : [m=1024, n=1024] per head, m on partitions -> softmax
                      denominator computed by PE via the ones column; exp on ACT
    U = [v|1]^T E   : [65, 1024] psum; row 64 = sum_m exp(scores^T[m, n])
    O = U[0:64] / S : normalize via base-0 S hop + reciprocal + gpsimd bcast
    y = WoutT^T O + bout' + x_self,  bout' = bout + Wout@bv (folded on host)

bk is dropped: it shifts every score of a softmax row by the same constant
(softmax invariant). bv is folded into bout' because attention rows sum to 1.

Head pairs (2p, 2p+1) share Q/K row-tiles; their K=64 score matmuls are issued
back-to-back at partition bases 0/64 so the PE runs them concurrently in
disjoint row groups. The attention loop is software-pipelined one pair ahead:
while pair p's U matmuls accumulate (m-tile at a time), pair p+1's scores and
exps stream, keeping both PE and ACT dense.

Numerics knobs (env):
    ATT_MM = f32r | f32 | bf16   dtype of projection/score matmuls
    ATT_VE = bf16 | f32r | f32   dtype of V/E/O/out-proj matmul path
"""

import os
from contextlib import ExitStack

import ml_dtypes
import numpy as np

import concourse.bass as bass
import concourse.tile as tile
from concourse import bacc, mybir

# Problem dims (fixed by the harness problem)
B = 8
C = 512  # QUERY_DIM == CROSS_DIM == INNER
HEADS = 8
HD = 64
N = 1024  # 32*32 pixels
N_CORES = 8
HDP = HD + 1  # per-head cols in VT_aug (64 v-cols + 1 ones col)

F32 = mybir.dt.float32
F32R = mybir.dt.float32r
BF16 = mybir.dt.bfloat16


def _storage(dt_name):
    if dt_name == "bf16":
        return BF16
    if dt_name == "f32r":
        return F32R
    return F32


def _np_storage(dt_name):
    return ml_dtypes.bfloat16 if dt_name == "bf16" else np.float32


def build(mm="f32r", ve="bf16"):
    nc = bacc.Bacc(
        "TRN2", target_bir_lowering=False, debug=False, num_devices=N_CORES
    )
    mdt = _storage(mm)  # x, Wq/Wk/Wv, Q, K storage
    vdt = _storage(ve)  # VT_aug, E, O, WoutT storage

    xs_d = nc.dram_tensor("x_self", [C, N], mdt, kind="ExternalInput").ap()
    xc_d = nc.dram_tensor("x_cross", [C, N], mdt, kind="ExternalInput").ap()
    wq_d = nc.dram_tensor("wqT", [C, C], mdt, kind="ExternalInput").ap()
    wk_d = nc.dram_tensor("wkT", [C, C], mdt, kind="ExternalInput").ap()
    wv_d = nc.dram_tensor("wvT", [C, C], mdt, kind="ExternalInput").ap()
    wo_d = nc.dram_tensor("woutT", [C, C], vdt, kind="ExternalInput").ap()
    bq_d = nc.dram_tensor("bq", [C], F32, kind="ExternalInput").ap()
    bo_d = nc.dram_tensor("bout2", [C], F32, kind="ExternalInput").ap()
    need_resid = mm == "bf16"
    if need_resid:
        rs_d = nc.dram_tensor("resid", [C, N], F32, kind="ExternalInput").ap()
    y_d = nc.dram_tensor("y", [C, N], F32, kind="ExternalOutput").ap()

    MUL = mybir.AluOpType.mult
    ADD = mybir.AluOpType.add
    EXP = mybir.ActivationFunctionType.Exp

    with tile.TileContext(nc) as tc, ExitStack() as ctx:
        persist = ctx.enter_context(tc.tile_pool(name="persist", bufs=1))
        ppool = ctx.enter_context(tc.tile_pool(name="psum", bufs=1, space="PSUM"))
        epool = ctx.enter_context(
            tc.tile_pool(name="epool", bufs=24 if vdt == BF16 else 9)
        )
        npool = ctx.enter_context(tc.tile_pool(name="norm", bufs=2))
        ypool = ctx.enter_context(tc.tile_pool(name="yout", bufs=2))

        def load(name, src, shape, dtype):
            t = persist.tile(shape, dtype, tag=name, name=name)
            nc.sync.dma_start(t[:], src)
            return t

        # ---- persistent loads (Q-projection-critical tensors first) ------
        wq_s = [load(f"wq{i}", wq_d[i * 128 : (i + 1) * 128, :], [128, C], mdt)
                for i in range(4)]
        xs_s = [load(f"xs{i}", xs_d[i * 128 : (i + 1) * 128, :], [128, N], mdt)
                for i in range(4)]
        wk_s = [load(f"wk{i}", wk_d[i * 128 : (i + 1) * 128, :], [128, C], mdt)
                for i in range(4)]
        xc_s = [load(f"xc{i}", xc_d[i * 128 : (i + 1) * 128, :], [128, N], mdt)
                for i in range(4)]
        wv_s = [load(f"wv{i}", wv_d[i * 128 : (i + 1) * 128, :], [128, C], mdt)
                for i in range(4)]
        bq_s = load("bq", bq_d.rearrange("(a p) -> p a", p=128), [128, 4], F32)
        wo_s = [load(f"wo{i}", wo_d[i * 128 : (i + 1) * 128, :], [128, C], vdt)
                for i in range(4)]
        bo_s = load("bo", bo_d.rearrange("(a p) -> p a", p=128), [128, 4], F32)
        if need_resid:
            rs_s = [load(f"rs{i}", rs_d[i * 128 : (i + 1) * 128, :], [128, N], F32)
                    for i in range(4)]
        elif mm == "f32r":
            # f32r storage holds full fp32 bits; view as fp32 for the residual
            rs_s = None
        else:
            rs_s = xs_s

        # VT_aug tiles: per-head [64 v-cols | ones] blocks
        vt_s = [persist.tile([128, HEADS * HDP], vdt, tag=f"vt{t}", name=f"vt{t}")
                for t in range(8)]
        for t in range(8):
            ones_cols = vt_s[t][:].rearrange("p (h d) -> p h d", d=HDP)[
                :, :, HD : HD + 1
            ]
            nc.gpsimd.memset(ones_cols, 1.0)

        # ---- Q / K projections ------------------------------------------
        q_s, k_s = [], []
        for which, w_s, x_in, dst in (("q", wq_s, xs_s, q_s), ("k", wk_s, xc_s, k_s)):
            for mt in range(4):
                ps = ppool.tile([128, N], F32, tag="u", name="ps", bufs=2)
                for nh in range(2):
                    for ct in range(4):
                        nc.tensor.matmul(
                            ps[:, nh * 512 : (nh + 1) * 512],
                            lhsT=w_s[ct][:, mt * 128 : (mt + 1) * 128],
                            rhs=x_in[ct][:, nh * 512 : (nh + 1) * 512],
                            start=(ct == 0),
                            stop=(ct == 3),
                        )
                t = persist.tile([128, N], mdt, tag=f"{which}{mt}",
                                 name=f"{which}{mt}")
                if which == "q":
                    nc.vector.tensor_scalar_add(t[:], ps[:], bq_s[:, mt : mt + 1])
                else:
                    nc.vector.tensor_copy(out=t[:], in_=ps[:])
                dst.append(t)

        # ---- V^T projection (x_cross^T @ Wv^T), emitted per m-tile ------
        def vt_proj_step(t):
            ps = ppool.tile([128, 512], F32, tag="u", name="vps", bufs=2)
            for ct in range(4):
                nc.tensor.matmul(
                    ps[:, 0:512],
                    lhsT=xc_s[ct][:, t * 128 : (t + 1) * 128],
                    rhs=wv_s[ct][:, :],
                    start=(ct == 0),
                    stop=(ct == 3),
                )
            vsrc = ps[:, 0:512].rearrange("p (h d) -> p h d", d=HD)
            vdst = vt_s[t][:].rearrange("p (h d) -> p h d", d=HDP)[:, :, 0:HD]
            nc.vector.tensor_copy(out=vdst, in_=vsrc)

        # ---- attention: pair-pipelined ----------------------------------
        o_s = [persist.tile([128, N], vdt, tag=f"o{i}", name=f"o{i}")
               for i in range(4)]

        def scores_step(p, t):
            """scores + exp for heads (2p, 2p+1) at m-tile t.
            Each n-half gets ONE psum tile holding both heads (a -> bank 0,
            b -> bank 1); the two K=64 matmuls sit at partition bases 0/64 and
            are issued back-to-back so the PE overlaps them in disjoint row
            groups, and one exp covers both. Returns E tiles [(t,nh)] with
            head a in cols 0:512 and head b in cols 512:1024."""
            es = []
            for nh in range(2):
                ps = ppool.tile([128, N], F32, tag="sc", name="sc", bufs=2)
                for i, base in enumerate((0, 64)):
                    nc.tensor.matmul(
                        ps[:, i * 512 : (i + 1) * 512],
                        lhsT=k_s[p][base : base + 64, t * 128 : (t + 1) * 128],
                        rhs=q_s[p][base : base + 64, nh * 512 : (nh + 1) * 512],
                        start=True,
                        stop=True,
                    )
                e = epool.tile([128, N], vdt, tag="e", name="e")
                nc.scalar.activation(e[:], ps[:], EXP, scale=0.125)
                es.append(e)
            return es

        def u_step(p, ups_pair, e_pair, t):
            """U accumulation for heads (2p, 2p+1) over m-tile t.
            e_pair[t] = (E_nh0, E_nh1); head a in cols 0:512, b in 512:1024."""
            for i, h in enumerate((2 * p, 2 * p + 1)):
                for nh in range(2):
                    nc.tensor.matmul(
                        ups_pair[i][0:65, nh * 512 : (nh + 1) * 512],
                        lhsT=vt_s[t][:, h * HDP : (h + 1) * HDP],
                        rhs=e_pair[t][nh][:, i * 512 : (i + 1) * 512],
                        start=(t == 0),
                        stop=(t == 7),
                    )

        def normalize(p, ups_pair):
            """O_h = U[0:64] / broadcast(U[64]) for heads (2p, 2p+1)."""
            for i, h in enumerate((2 * p, 2 * p + 1)):
                ups = ups_pair[i]
                base = (h % 2) * 64
                s0 = npool.tile([1, N], F32, tag="s0", name="s0", bufs=1)
                nc.vector.tensor_copy(out=s0[:], in_=ups[64:65, :])
                r0 = npool.tile([1, N], F32, tag="r0", name="r0", bufs=1)
                nc.vector.reciprocal_approx_fast(r0[:], s0[:])
                R = npool.tile([64, N], F32, tag="R", name="R")
                nc.gpsimd.partition_broadcast(R[:], r0[:])
                if base == 0:
                    nc.vector.tensor_tensor(
                        o_s[p][0:64, :], ups[0:64, :], R[:], op=MUL
                    )
                else:
                    stg = npool.tile([64, N], vdt, tag="stg", name="stg")
                    nc.vector.tensor_tensor(stg[:], ups[0:64, :], R[:], op=MUL)
                    nc.vector.tensor_copy(out=o_s[p][64:128, :], in_=stg[:])

        # prologue: scores for pair 0, with the V^T projection interleaved
        # into the exp-paced gaps
        e_prev = []
        for t in range(8):
            e_prev.append(scores_step(0, t))
            vt_proj_step(t)

        for p in range(4):
            ups_pair = [
                ppool.tile([128, N], F32, tag="u", name="ua", bufs=2),
                ppool.tile([128, N], F32, tag="u", name="ub", bufs=2),
            ]
            e_next = []
            for t in range(8):
                if p + 1 < 4:
                    e_next.append(scores_step(p + 1, t))
                u_step(p, ups_pair, e_prev, t)
            normalize(p, ups_pair)
            e_prev = e_next

        # ---- output projection + bias + residual ------------------------
        for mt in range(4):
            ps = ppool.tile([128, N], F32, tag="u", name="ps", bufs=2)
            for nh in range(2):
                for dt_ in range(4):
                    nc.tensor.matmul(
                        ps[:, nh * 512 : (nh + 1) * 512],
                        lhsT=wo_s[dt_][:, mt * 128 : (mt + 1) * 128],
                        rhs=o_s[dt_][:, nh * 512 : (nh + 1) * 512],
                        start=(dt_ == 0),
                        stop=(dt_ == 3),
                    )
            y = ypool.tile([128, N], F32, tag="y", name="y_t")
            if rs_s is None:
                resid_ap = xs_s[mt][:].bitcast(F32)
            else:
                resid_ap = rs_s[mt][:]
            nc.vector.scalar_tensor_tensor(
                y[:], ps[:], bo_s[:, mt : mt + 1], resid_ap, op0=ADD, op1=ADD
            )
            nc.sync.dma_start(y_d[mt * 128 : (mt + 1) * 128, :], y[:])

    nc.compile()
    return nc


_CACHE = {}


def get_nc(mm=None, ve=None):
    mm = mm or os.environ.get("ATT_MM", "f32r")
    ve = ve or os.environ.get("ATT_VE", "bf16")
    key = (mm, ve)
    if key not in _CACHE:
        _CACHE[key] = build(*key)
    return _CACHE[key], key


def make_in_maps(self_feature, cross_feature, Wq, bq, Wk, bk, Wv, bv, Wout, bout,
                 mm, ve):
    f32 = np.float32
    np_m = _np_storage(mm)
    np_v = _np_storage(ve)
    sf = np.asarray(self_feature, f32).reshape(B, C, N)
    cf = np.asarray(cross_feature, f32).reshape(B, C, N)
    Wq = np.asarray(Wq, f32)
    Wk = np.asarray(Wk, f32)
    Wv = np.asarray(Wv, f32)
    Wout = np.asarray(Wout, f32)
    wqT = np.ascontiguousarray(Wq.T).astype(np_m)
    wkT = np.ascontiguousarray(Wk.T).astype(np_m)
    wvT = np.ascontiguousarray(Wv.T).astype(np_m)
    woT = np.ascontiguousarray(Wout.T).astype(np_v)
    bq = np.asarray(bq, f32)
    bout2 = (np.asarray(bout, f32) + Wout @ np.asarray(bv, f32)).astype(f32)
    # bk is intentionally unused: adding k-bias shifts all scores in a softmax
    # row by the same amount, which softmax cancels exactly.
    del bk
    in_maps = []
    for b in range(B):
        m = {
            "x_self": np.ascontiguousarray(sf[b]).astype(np_m),
            "x_cross": np.ascontiguousarray(cf[b]).astype(np_m),
            "wqT": wqT, "wkT": wkT, "wvT": wvT, "woutT": woT,
            "bq": bq, "bout2": bout2,
        }
        if mm == "bf16":
            m["resid"] = np.ascontiguousarray(sf[b])
        in_maps.append(m)
    return in_maps


def kernel(self_feature, cross_feature, Wq, bq, Wk, bk, Wv, bv, Wout, bout):
    from concourse.bass_utils import run_bass_kernel_spmd

    nc, (mm, ve) = get_nc()
    in_maps = make_in_maps(self_feature, cross_feature, Wq, bq, Wk, bk, Wv, bv,
                           Wout, bout, mm, ve)
    res = run_bass_kernel_spmd(nc, in_maps, core_ids=list(range(N_CORES)))
    y = np.stack([res.results[b]["y"].reshape(C, 32, 32) for b in range(B)])
    return np.ascontiguousarray(y.astype(np.float32))
